# revision 1
# baseline (speedup 1.0000x reference)
"""Trainium2 Bass kernel for causal multi-head attention (no KV cache).

Problem: x[1,4096,1024], w_qkv[3072,1024], w_out[1024,1024], 16 heads, hd=64.
Sharding: tensor-parallel over heads — each of the 8 cores owns 2 heads.
Per core: QKV projection for its heads, causal softmax(QK^T/8)V, and the
partial output projection  y_c @ w_out[:, c*128:(c+1)*128].T  -> [4096,1024].
Host sums the 8 fp32 partials.

Device layout choices:
  - x is shipped pre-transposed (xT [1024,4096], bf16): the QKV matmuls
    contract over the model dim, which must live on SBUF partitions.
  - q^T and k^T are produced directly as [64, T] (head dim on partitions) by
    making the projection-weight slice the stationary operand.  1/sqrt(hd) is
    folded into Wq on the host.
  - scores are computed transposed, S^T[k,q], so the exp output is already the
    lhsT the PV matmul needs — no on-device transposes in the inner loop.
  - softmax denominator comes free from a ones-column appended to V.
  - causal masking: k-tiles strictly above the diagonal are skipped, the
    diagonal tiles are free-dim-trimmed and masked with a static [128,128]
    triangular multiply.
"""

import os
import sys

import numpy as np

for _p in ("/opt/trn_rl_repo", "/root/.axon_site/_ro/trn_rl_repo"):
    if os.path.isdir(_p) and _p not in sys.path:
        sys.path.insert(0, _p)

import ml_dtypes

import concourse.bass as bass
import concourse.mybir as mybir
import concourse.tile as tile
from concourse import bacc, masks

BF16 = mybir.dt.bfloat16
F32 = mybir.dt.float32
NPBF16 = ml_dtypes.bfloat16

D_MODEL = 1024
N_HEADS = 16
HEAD_DIM = 64
N_CORES = 8
HEADS_PER_CORE = N_HEADS // N_CORES  # 2
T_FULL = 4096


def build_program(
    T: int = T_FULL, loop_reps: int | None = None, variant: str = "wide"
) -> bass.Bass:
    """One NeuronCore's program; all 8 cores run it on different data.

    loop_reps: wrap the whole kernel in a device-side For loop (identical
    iterations) — benchmarking aid: slope of wall-time over reps gives the
    per-iteration device time without host/tunnel dispatch noise.

    variant: psum allocation strategy.
      "deep": scores triple-buffered (3x2 banks), PV accumulators 2x1,
              projection transients share the scores pool.
      "wide": scores double-buffered (2x2), PV accumulators + transients
              share a 4x1-bank pool (two chunks of accumulators in flight).
    """
    assert T % 1024 == 0
    assert variant in ("deep", "wide")
    TT = T // 128          # 128-row T-tiles
    CH = D_MODEL // 128    # c-chunks of the contraction dim
    QC = T // 1024         # 1024-wide q chunks
    H2 = HEADS_PER_CORE

    nc = bacc.Bacc("TRN2", target_bir_lowering=False, debug=False)

    xt = nc.dram_tensor("xt", [D_MODEL, T], BF16, kind="ExternalInput").ap()
    # packed per c-chunk: 256 cols = [q_h0 | q_h1 | k_h0 | k_h1] (64 each)
    wqk = nc.dram_tensor("wqk", [128, CH * 256], BF16, kind="ExternalInput").ap()
    # packed per c-chunk: 128 cols = [v_h0 | v_h1]
    wv = nc.dram_tensor("wv", [128, CH * 128], BF16, kind="ExternalInput").ap()
    # w_out[:, c*128:(c+1)*128].T
    wo = nc.dram_tensor("wo", [128, D_MODEL], BF16, kind="ExternalInput").ap()
    # bf16 partials: halves the 16MB/core output traffic; the fp32 summation
    # happens on the host.  Measured accuracy cost: 0.44% -> 0.47% rel err.
    out = nc.dram_tensor("out_partial", [T, D_MODEL], BF16, kind="ExternalOutput").ap()

    with tile.TileContext(nc) as tc:
        import contextlib

        with contextlib.ExitStack() as ctx:
            const_pool = ctx.enter_context(tc.tile_pool(name="const", bufs=1))
            xt_pool = ctx.enter_context(tc.tile_pool(name="xt", bufs=1))
            qk_pool = ctx.enter_context(tc.tile_pool(name="qk", bufs=1))
            v_pool = ctx.enter_context(tc.tile_pool(name="v", bufs=1))
            y_pool = ctx.enter_context(tc.tile_pool(name="y", bufs=1))
            pt_pool = ctx.enter_context(tc.tile_pool(name="ptp", bufs=8))
            yt_pool = ctx.enter_context(tc.tile_pool(name="ytp", bufs=5))
            ob_pool = ctx.enter_context(tc.tile_pool(name="obp", bufs=4))
            rec_pool = ctx.enter_context(tc.tile_pool(name="recp", bufs=8))
            deep = variant == "deep"
            psum_mm = ctx.enter_context(
                tc.tile_pool(name="psmm", bufs=3 if deep else 2, space="PSUM")
            )
            psum_av = ctx.enter_context(
                tc.tile_pool(name="psav", bufs=2 if deep else 4, space="PSUM")
            )

            def trans_tile(name, w):
                """1-bank transient psum [128, w<=512] for projections."""
                if deep:
                    t = psum_mm.tile([128, 1024], F32, name=name, tag="mm1k")
                else:
                    t = psum_av.tile([128, 512], F32, name=name, tag="av")
                return t[:, 0:w]

            if loop_reps:
                ctx.enter_context(tc.For_i(0, loop_reps, 1))

            # --- constants ---
            # trimask[p, g] = 1.0 where p <= g else 0  (keep k <= q)
            trimask = const_pool.tile([128, 128], BF16, name="trimask")
            masks.make_upper_triangular(nc, trimask, val=1.0, diag=True)

            # --- x^T + weight loads.  The first attention chunk only needs
            # --- x columns [0,1024), so DMA T-slice by T-slice: attention
            # --- starts ~6us in instead of waiting for the full 8MB. ---
            xt_sb = []
            for cc in range(CH):
                t = xt_pool.tile([128, T], BF16, name=f"xt{cc}", tag=f"xt{cc}")
                xt_sb.append(t)
            wqk_sb = const_pool.tile([128, CH * 256], BF16, name="wqk_sb")
            nc.sync.dma_start(wqk_sb, wqk)
            # first 512 cols of x^T unblock the first q/k projection groups;
            # the rest streams behind
            for half in range(2):
                for cc in range(CH):
                    nc.sync.dma_start(
                        xt_sb[cc][:, half * 512:half * 512 + 512],
                        xt[cc * 128:(cc + 1) * 128, half * 512:half * 512 + 512],
                    )
            wv_sb = const_pool.tile([128, CH * 128], BF16, name="wv_sb")
            nc.sync.dma_start(wv_sb, wv)
            wo_sb = const_pool.tile([128, D_MODEL], BF16, name="wo_sb")
            nc.sync.dma_start(wo_sb, wo)

            def emit_xt_dma(tch):
                for cc in range(CH):
                    nc.sync.dma_start(
                        xt_sb[cc][:, tch * 1024:(tch + 1) * 1024],
                        xt[cc * 128:(cc + 1) * 128, tch * 1024:(tch + 1) * 1024],
                    )

            # --- persistent tiles ---
            q_sb = qk_pool.tile([128, T], BF16, name="q_sb", tag="q_sb")
            k_sb = qk_pool.tile([128, T], BF16, name="k_sb", tag="k_sb")
            v_sb = [[None] * TT for _ in range(H2)]
            y_sb = []
            for qt in range(TT):
                t = y_pool.tile([128, 128], BF16, name=f"y{qt}", tag=f"y{qt}")
                y_sb.append(t)

            # --- emission helpers (deferred so phases interleave on PE) ---
            _qkv_boxes = {}

            def emit_qkv_part(g, tch, half, part):
                """q^T (g=0) or k^T (g=1) for cols [1024*tch+512*half, +512).
                Split into two 4-matmul parts so one background pop doesn't
                monopolize PE long enough to starve the exp stream."""
                key = (g, tch, half)
                c0 = tch * 1024 + half * 512
                if part == 0:
                    _qkv_boxes[key] = trans_tile(f"qkps{g}_{tch}_{half}", 512)
                ps = _qkv_boxes[key]
                for cc in range(part * 4, part * 4 + 4):
                    nc.tensor.matmul(
                        ps,
                        lhsT=wqk_sb[:, cc * 256 + g * 128: cc * 256 + g * 128 + 128],
                        rhs=xt_sb[cc][:, c0:c0 + 512],
                        start=(cc == 0),
                        stop=(cc == CH - 1),
                    )
                if part == 1:
                    dst = q_sb if g == 0 else k_sb
                    nc.vector.tensor_copy(dst[:, c0:c0 + 512], ps)
                    del _qkv_boxes[key]

            def emit_qkv_group(g, tch, half):
                emit_qkv_part(g, tch, half, 0)
                emit_qkv_part(g, tch, half, 1)

            v_pending = {}  # kt -> deferred emit, forced before first use

            def ensure_v(kt):
                fn = v_pending.pop(kt, None)
                if fn is not None:
                    fn()

            _v_boxes = {}

            def emit_v_part(kt, part):
                """v[k-tile, 64+ones] for both heads; two 4-matmul parts."""
                if part == 0:
                    _v_boxes[kt] = trans_tile(f"vps{kt}", 128)
                vps = _v_boxes[kt]
                for cc in range(part * 4, part * 4 + 4):
                    nc.tensor.matmul(
                        vps,
                        lhsT=xt_sb[cc][:, kt * 128:(kt + 1) * 128],
                        rhs=wv_sb[:, cc * 128:(cc + 1) * 128],
                        start=(cc == 0),
                        stop=(cc == CH - 1),
                    )
                if part == 1:
                    for h in range(H2):
                        vt = v_pool.tile([128, 65], BF16, name=f"v{h}_{kt}", tag=f"v{h}_{kt}")
                        nc.vector.tensor_copy(vt[:, 0:64], vps[:, h * 64:(h + 1) * 64])
                        nc.vector.memset(vt[:, 64:65], 1.0)
                        v_sb[h][kt] = vt
                    del _v_boxes[kt]

            def emit_v_tile(kt):
                emit_v_part(kt, 0)
                emit_v_part(kt, 1)

            def emit_outproj_batch(qts, tail=False):
                """Batched projection: all transposes first (one DMA-xbar
                transition), then the matmul/copy/store pipeline."""
                yts = []
                for qt in qts:
                    yt = yt_pool.tile([128, 128], BF16, name=f"yt{qt}", tag="yt")
                    nc.sync.dma_start(yt, y_sb[qt], transpose=True)
                    yts.append(yt)
                for qt, yt in zip(qts, yts):
                    emit_outproj_mm(qt, yt, tail)

            def emit_outproj(qt):
                """partial[q-tile, :] = y^T via DMA transpose, then 2 matmuls."""
                yt = yt_pool.tile([128, 128], BF16, name=f"yt{qt}", tag="yt")
                nc.sync.dma_start(yt, y_sb[qt], transpose=True)
                emit_outproj_mm(qt, yt)

            def emit_outproj_mm(qt, yt, tail=False):
                # kernel tail: ACT is done with exps -> evacuate psum there
                cp = nc.scalar.copy if tail else nc.vector.tensor_copy
                if deep:
                    ops = psum_mm.tile([128, 1024], F32, name=f"op{qt}", tag="mm1k")
                    for n2 in range(D_MODEL // 512):
                        nc.tensor.matmul(
                            ops[:, n2 * 512:(n2 + 1) * 512],
                            lhsT=yt,
                            rhs=wo_sb[:, n2 * 512:(n2 + 1) * 512],
                            start=True,
                            stop=True,
                        )
                    ob = ob_pool.tile([128, 1024], BF16, name=f"ob{qt}", tag="ob")
                    cp(ob, ops)
                    nc.sync.dma_start(out[qt * 128:(qt + 1) * 128, :], ob)
                else:
                    for n2 in range(D_MODEL // 512):
                        ops = trans_tile(f"op{qt}_{n2}", 512)
                        nc.tensor.matmul(
                            ops,
                            lhsT=yt,
                            rhs=wo_sb[:, n2 * 512:(n2 + 1) * 512],
                            start=True,
                            stop=True,
                        )
                        ob = ob_pool.tile([128, 512], BF16, name=f"ob{qt}_{n2}", tag="ob")
                        cp(ob, ops)
                        nc.sync.dma_start(
                            out[qt * 128:(qt + 1) * 128, n2 * 512:(n2 + 1) * 512], ob
                        )

            # --- preamble: projections needed by the first attention step ---
            if T > 1024:
                emit_xt_dma(1)
            # warm the PE clock (HAM releases the 1.2GHz throttle after
            # ~3.4us of sustained activity) with throwaway matmuls on the
            # mask tile while the input DMAs are still in flight
            warm = trans_tile("warmps", 128)
            for i in range(34):
                nc.tensor.matmul(warm, lhsT=trimask, rhs=trimask,
                                 start=True, stop=True)
            # half-major: the first scores piece needs q AND k of cols
            # [0,512) before either half-1 group
            for half in range(2):
                for g in range(2):
                    emit_qkv_group(g, 0, half)
            emit_v_tile(0)

            # --- attention, software-pipelined; other phases fed in as
            # --- background PE work so no engine sits idle ---
            from collections import deque

            EXP = mybir.ActivationFunctionType.Exp
            bg = deque()
            op_pending = deque()  # output-projection work ready to schedule

            def queue_v(kt):
                v_pending[kt] = lambda: emit_v_tile(kt)
                bg.append(lambda: ensure_v(kt))

            for Q in range(QC):
                if Q + 2 < QC:
                    bg.append(lambda t=Q + 2: emit_xt_dma(t))
                # V tiles for THIS chunk's k range: only needed from the
                # iteration matching their kt (ensure_v forces stragglers)
                for kt in range(8 * Q, 8 * Q + 8):
                    if kt > 0:
                        queue_v(kt)
                if Q + 1 < QC:
                    # q/k for the next chunk: hard deadline at its start
                    for g in range(2):
                        for half in range(2):
                            for part in range(2):
                                bg.append(
                                    lambda g=g, t=Q + 1, hf=half, p=part: emit_qkv_part(g, t, hf, p)
                                )
                if Q > 0:
                    for s in range(8):
                        op_pending.append(lambda qt=8 * (Q - 1) + s: emit_outproj(qt))
                # early chunks are PE-bound (projection deadlines for the next
                # chunk); late chunks are exp-bound with idle PE — so place
                # the deferrable output-projection work late
                if Q == QC - 1:
                    bg.extend(op_pending)
                    op_pending.clear()
                elif Q == QC - 2:
                    for _ in range(min(4, len(op_pending))):
                        bg.append(op_pending.popleft())
                iters = 2 * (8 * Q + 8)
                if Q == QC - 1:
                    # last chunk: no later chunk can absorb stragglers, so
                    # drain the background queue by ~60% through
                    iters = max(1, (iters * 3) // 5)
                bg_total = len(bg)
                bg_emitted = 0
                it_idx = 0

                for h in range(H2):
                    hp = h * 64  # partition base of this head in q_sb/k_sb
                    # tail chunk: stream epilogue+projection out of the loop
                    # as each q-subtile's accumulation finishes, so the kernel
                    # doesn't end with a long serial projection phase
                    streaming = (Q == QC - 1) and (h == H2 - 1)
                    done_s = set()

                    # q-subtiles packed into 1-bank accumulators, col layout
                    # [65 x n]: cols 0-63 = PV, col 64 = sum(exp).  The
                    # normal split is {0-3},{4-7}; the very last head splits
                    # the high bank further so its subtiles drain (and their
                    # output projection starts) before the final PV matmul.
                    # smap: s -> (tile idx, col idx, tile's stop subtile)
                    groups = [(0, 1, 2, 3), (4, 5, 6, 7)]
                    tiles = [
                        psum_av.tile([128, 65 * len(g)], F32,
                                     name=f"av{h}_{Q}_{ti}", tag="av")
                        for ti, g in enumerate(groups)
                    ]
                    smap = {}
                    for ti, g in enumerate(groups):
                        for ci, s in enumerate(g):
                            smap[s] = (ti, ci, g[-1])

                    def emit_epilogue(s, h=h, hp=hp, Q=Q, tiles=tiles, smap=smap):
                        qt = 8 * Q + s
                        ti, ci, _ = smap[s]
                        avt = tiles[ti]
                        col = ci * 65
                        rec = rec_pool.tile([128, 1], F32, name=f"rec{h}_{qt}", tag="rec")
                        nc.vector.reciprocal(rec, avt[:, col + 64:col + 65])
                        nc.vector.tensor_scalar_mul(
                            y_sb[qt][:, hp:hp + 64], avt[:, col:col + 64], rec
                        )
                    pending = []  # AV matmuls delayed two iterations: keeps
                    # PE from stalling on exp(kt) before issuing later scores
                    pending_old = []
                    for kt in range(8 * Q + 8):
                        i = kt - 8 * Q  # >= 0 on diagonal tiles
                        off = 128 * i if i > 0 else 0
                        sps = psum_mm.tile([128, 1024], F32, name=f"s{h}_{Q}_{kt}", tag="mm1k")
                        bounds = [(off, 512), (512, 1024)] if off < 512 else [(off, 1024)]
                        for (a, b) in bounds:
                            nc.tensor.matmul(
                                sps[:, a:b],
                                lhsT=k_sb[hp:hp + 64, kt * 128:(kt + 1) * 128],
                                rhs=q_sb[hp:hp + 64, Q * 1024 + a: Q * 1024 + b],
                                start=True,
                                stop=True,
                            )
                        pt = pt_pool.tile([128, 1024], BF16, name=f"pt{h}_{Q}_{kt}", tag="pt")
                        if Q == 0 and h == 0:
                            # pipeline fill: exp per scores piece so ACT
                            # starts after only half the preamble DMA
                            for (a, b) in bounds:
                                nc.scalar.activation(pt[:, a:b], sps[:, a:b], EXP)
                        else:
                            nc.scalar.activation(pt[:, off:1024], sps[:, off:1024], EXP)
                        if i >= 0:
                            nc.vector.tensor_mul(
                                pt[:, off:off + 128], pt[:, off:off + 128], trimask
                            )
                        ensure_v(kt)  # force the deferred V tile if the
                        # background spread hasn't reached it yet
                        for fn in pending_old:
                            fn()
                        pending_old = pending
                        pending = []
                        if kt - 2 - 8 * Q >= 0:
                            # drain accumulator tiles whose group closed at
                            # kt-1 (stop fires on the tile's last subtile):
                            # legal in every chunk — the bank gets no further
                            # writes — and frees its psum slot 4 iterations
                            # early; the tail head also projects right away
                            sdone = kt - 2 - 8 * Q
                            drained = [
                                s for s in range(8)
                                if smap[s][2] == sdone and s not in done_s
                            ]
                            for s in drained:
                                done_s.add(s)
                                emit_epilogue(s)
                            if streaming and drained:
                                emit_outproj_batch([8 * Q + s for s in drained])
                        for s in range(max(i, 0), 8):
                            # start/stop are per 2KB psum zero-region (= one
                            # bank): first/last matmul touching each packed
                            # accumulator tile, not each 65-col slice
                            ti, ci, stop_s = smap[s]
                            pending.append(
                                lambda pt=pt, s=s, kt=kt, avt=tiles[ti], col=ci * 65, stop_s=stop_s: nc.tensor.matmul(
                                    avt[:, col:col + 65],
                                    lhsT=pt[:, s * 128:s * 128 + 128],
                                    rhs=v_sb[h][kt],
                                    start=(kt == 0 and col == 0),
                                    stop=(kt == 8 * Q + s and s == stop_s),
                                )
                            )
                        it_idx += 1
                        while bg and bg_emitted < (it_idx * bg_total + iters - 1) // iters:
                            bg.popleft()()
                            bg_emitted += 1
                    for fn in pending_old:
                        fn()
                    for fn in pending:
                        fn()
                    left = [s for s in range(8) if s not in done_s]
                    for s in left:
                        emit_epilogue(s)
                    if streaming and left:
                        emit_outproj_batch([8 * Q + s for s in left], tail=True)
                while bg:
                    bg.popleft()()

    nc.compile()
    return nc


def make_in_maps(x, w_qkv, w_out, T: int = T_FULL):
    """Shard full inputs into the 8 per-core input dicts."""
    x = np.asarray(x, dtype=np.float32)
    w_qkv = np.asarray(w_qkv, dtype=np.float32)
    w_out = np.asarray(w_out, dtype=np.float32)
    xm = x.reshape(-1, D_MODEL)[:T]  # [T, C]
    xt = np.ascontiguousarray(xm.T).astype(NPBF16)  # [C, T]

    CH = D_MODEL // 128
    Wq = w_qkv[0:D_MODEL] * np.float32(1.0 / np.sqrt(HEAD_DIM))
    Wk = w_qkv[D_MODEL:2 * D_MODEL]
    Wv = w_qkv[2 * D_MODEL:3 * D_MODEL]

    in_maps = []
    for c in range(N_CORES):
        r0 = c * 128
        # [256 rows, C] = [q_h0 | q_h1 | k_h0 | k_h1] stacked along rows
        qk_rows = np.concatenate(
            [Wq[r0:r0 + 128], Wk[r0:r0 + 128]], axis=0
        )  # [256, C]
        # -> [C, 256] -> packed [128, CH*256]
        qk_t = qk_rows.T.reshape(CH, 128, 256).transpose(1, 0, 2).reshape(128, CH * 256)
        v_rows = Wv[r0:r0 + 128]  # [128, C] = [v_h0 | v_h1] along rows
        v_t = v_rows.T.reshape(CH, 128, 128).transpose(1, 0, 2).reshape(128, CH * 128)
        wo_t = np.ascontiguousarray(w_out[:, r0:r0 + 128].T)  # [128, C]
        in_maps.append(
            {
                "xt": xt,
                "wqk": np.ascontiguousarray(qk_t).astype(NPBF16),
                "wv": np.ascontiguousarray(v_t).astype(NPBF16),
                "wo": wo_t.astype(NPBF16),
            }
        )
    return in_maps


_program_cache = {}


def get_program(
    T: int = T_FULL, loop_reps: int | None = None, variant: str = "wide"
) -> bass.Bass:
    key = (T, loop_reps, variant)
    if key not in _program_cache:
        _program_cache[key] = build_program(T, loop_reps, variant)
    return _program_cache[key]


def run_on_hw(x, w_qkv, w_out, trace: bool = False, T: int = T_FULL):
    from concourse.bass_utils import run_bass_kernel_spmd

    nc = get_program(T)
    in_maps = make_in_maps(x, w_qkv, w_out, T)
    res = run_bass_kernel_spmd(nc, in_maps, core_ids=list(range(N_CORES)), trace=trace)
    acc = np.zeros((T, D_MODEL), np.float32)
    for c in range(N_CORES):
        acc += np.asarray(res.results[c]["out_partial"], dtype=np.float32)
    return acc.reshape(1, T, D_MODEL), res


def kernel(x, w_qkv, w_out):
    out, _ = run_on_hw(x, w_qkv, w_out)
    return out.astype(np.float32)



# revision 2
# speedup vs baseline: 1.1641x; 1.1641x over previous
"""Trainium2 Bass kernel for causal MHA (no KV cache), v3.

v3 changes (driven by HW ablation of v2):
  - exp instructions carry ~290ns of un-pipelined overhead each (sem-wait
    breaks the ACT pipeline), so halve their count: scores for a PAIR of
    k-tiles land in one [128,1024] psum tile per head (cols 0-511 = kt,
    512-1023 = kt+1 -- same q range, different k rows) and one exp covers
    both.  The two heads' tiles are single-buffered; the heads alternate on
    ACT so each head's tile is free again by the time its next scores pair
    issues (mutual ping-pong).
  - output projection: psum evacuation alternates DVE/ACT halves (GPSIMD
    cannot touch PSUM); the causal-mask multiplies (SBUF-only) run on the
    otherwise-idle GpSimd engine instead of DVE.
  - V tiles stored [v0 | ones | v1] ([128,129]): head0 reads cols 0-64,
    head1 cols 64-128 (ones shared, head1's denominator lands in col 0 of
    its PV output); built with ONE strided-dst copy + one memset.
  - scores K=64 matmuls of the two heads are adjacent -> different PE row
    groups run concurrently (v2's win, kept).
"""

import os
import sys

import numpy as np

for _p in ("/opt/trn_rl_repo", "/root/.axon_site/_ro/trn_rl_repo"):
    if os.path.isdir(_p) and _p not in sys.path:
        sys.path.insert(0, _p)

import ml_dtypes

import concourse.bass as bass
import concourse.mybir as mybir
import concourse.tile as tile
from concourse import bacc, masks

BF16 = mybir.dt.bfloat16
F32 = mybir.dt.float32
NPBF16 = ml_dtypes.bfloat16

D_MODEL = 1024
N_HEADS = 16
HEAD_DIM = 64
N_CORES = 8
HEADS_PER_CORE = N_HEADS // N_CORES  # 2
T_FULL = 4096
QW = 512  # q-chunk width


def build_program(T: int = T_FULL, loop_reps: int | None = None) -> bass.Bass:
    assert T % QW == 0
    TT = T // 128          # 128-row t-tiles
    CH = D_MODEL // 128    # c-chunks of the contraction dim
    QC = T // QW           # q chunks
    SPC = QW // 128        # q-subtiles per chunk (4)
    KPC = QW // 128        # new k-tiles per chunk (4)

    nc = bacc.Bacc("TRN2", target_bir_lowering=False, debug=False)

    xt = nc.dram_tensor("xt", [D_MODEL, T], BF16, kind="ExternalInput").ap()
    wqk = nc.dram_tensor("wqk", [128, CH * 256], BF16, kind="ExternalInput").ap()
    wv = nc.dram_tensor("wv", [128, CH * 128], BF16, kind="ExternalInput").ap()
    wo = nc.dram_tensor("wo", [128, D_MODEL], BF16, kind="ExternalInput").ap()
    out = nc.dram_tensor("out_partial", [T, D_MODEL], BF16, kind="ExternalOutput").ap()

    with tile.TileContext(nc) as tc:
        import contextlib
        from collections import deque

        EXP = mybir.ActivationFunctionType.Exp

        with contextlib.ExitStack() as ctx:
            const_pool = ctx.enter_context(tc.tile_pool(name="const", bufs=1))
            xt_pool = ctx.enter_context(tc.tile_pool(name="xt", bufs=1))
            qk_pool = ctx.enter_context(tc.tile_pool(name="qk", bufs=1))
            v_pool = ctx.enter_context(tc.tile_pool(name="v", bufs=1))
            y_pool = ctx.enter_context(tc.tile_pool(name="y", bufs=1))
            pt_pool = ctx.enter_context(tc.tile_pool(name="ptp", bufs=4))
            yt_pool = ctx.enter_context(tc.tile_pool(name="ytp", bufs=5))
            ob_pool = ctx.enter_context(tc.tile_pool(name="obp", bufs=4))
            rec_pool = ctx.enter_context(tc.tile_pool(name="recp", bufs=4))
            # scores: [128,1024] (2 banks) per head, single-buffered -> 4
            ps_s = ctx.enter_context(tc.tile_pool(name="pss", bufs=1, space="PSUM"))
            # PV accumulators: one bank per head -> 2
            ps_av = ctx.enter_context(tc.tile_pool(name="psav", bufs=1, space="PSUM"))
            # projection transients -> 2
            ps_t = ctx.enter_context(tc.tile_pool(name="pst", bufs=2, space="PSUM"))

            if loop_reps:
                ctx.enter_context(tc.For_i(0, loop_reps, 1))

            # --- constants ---
            trimask = const_pool.tile([128, 128], BF16, name="trimask")
            masks.make_upper_triangular(nc, trimask, val=1.0, diag=True)

            # --- x^T + weight loads ---
            xt_sb = []
            for cc in range(CH):
                t = xt_pool.tile([128, T], BF16, name=f"xt{cc}", tag=f"xt{cc}")
                xt_sb.append(t)
            wqk_sb = const_pool.tile([128, CH * 256], BF16, name="wqk_sb")
            nc.sync.dma_start(wqk_sb, wqk)
            for half in range(2):
                for cc in range(CH):
                    nc.sync.dma_start(
                        xt_sb[cc][:, half * 512:half * 512 + 512],
                        xt[cc * 128:(cc + 1) * 128, half * 512:half * 512 + 512],
                    )
            wv_sb = const_pool.tile([128, CH * 128], BF16, name="wv_sb")
            nc.sync.dma_start(wv_sb, wv)
            wo_sb = const_pool.tile([128, D_MODEL], BF16, name="wo_sb")
            nc.sync.dma_start(wo_sb, wo)

            def emit_xt_dma(tch):
                for cc in range(CH):
                    nc.sync.dma_start(
                        xt_sb[cc][:, tch * 512:(tch + 1) * 512],
                        xt[cc * 128:(cc + 1) * 128, tch * 512:(tch + 1) * 512],
                    )

            # --- persistent tiles ---
            q_sb = qk_pool.tile([128, T], BF16, name="q_sb", tag="q_sb")
            k_sb = qk_pool.tile([128, T], BF16, name="k_sb", tag="k_sb")
            v_sb = [None] * TT          # [128,129] = [v0 | ones | v1]
            y_sb = []
            for qt in range(TT):
                t = y_pool.tile([128, 128], BF16, name=f"y{qt}", tag=f"y{qt}")
                y_sb.append(t)

            # --- deferred emission helpers ---
            _qkv_boxes = {}

            def emit_qkv_part(g, tch, part):
                key = (g, tch)
                c0 = tch * 512
                if part == 0:
                    _qkv_boxes[key] = ps_t.tile([128, 512], F32,
                                                name=f"qk{g}_{tch}", tag="tr")
                ps = _qkv_boxes[key]
                for cc in range(part * 4, part * 4 + 4):
                    nc.tensor.matmul(
                        ps,
                        lhsT=wqk_sb[:, cc * 256 + g * 128: cc * 256 + g * 128 + 128],
                        rhs=xt_sb[cc][:, c0:c0 + 512],
                        start=(cc == 0),
                        stop=(cc == CH - 1),
                    )
                if part == 1:
                    dst = q_sb if g == 0 else k_sb
                    nc.vector.tensor_copy(dst[:, c0:c0 + 512], ps)
                    del _qkv_boxes[key]

            v_pending = {}
            _v_boxes = {}

            def emit_v_part(kt, part):
                if part == 0:
                    _v_boxes[kt] = ps_t.tile([128, 512], F32,
                                             name=f"vps{kt}", tag="tr")
                vps = _v_boxes[kt]
                for cc in range(part * 4, part * 4 + 4):
                    nc.tensor.matmul(
                        vps[:, 0:128],
                        lhsT=xt_sb[cc][:, kt * 128:(kt + 1) * 128],
                        rhs=wv_sb[:, cc * 128:(cc + 1) * 128],
                        start=(cc == 0),
                        stop=(cc == CH - 1),
                    )
                if part == 1:
                    vt = v_pool.tile([128, 129], BF16, name=f"v{kt}",
                                     tag=f"v{kt}")
                    nc.vector.tensor_copy(vt[:, 0:64], vps[:, 0:64])
                    nc.vector.tensor_copy(vt[:, 65:129], vps[:, 64:128])
                    nc.vector.memset(vt[:, 64:65], 1.0)
                    v_sb[kt] = vt
                    del _v_boxes[kt]

            def emit_v_tile(kt):
                emit_v_part(kt, 0)
                emit_v_part(kt, 1)

            def ensure_v(kt):
                fn = v_pending.pop(kt, None)
                if fn is not None:
                    fn()

            def queue_v(kt):
                v_pending[kt] = lambda: emit_v_tile(kt)
                bg.append(lambda: ensure_v(kt))

            def emit_outproj(qt, tail=False):
                yt = yt_pool.tile([128, 128], BF16, name=f"yt{qt}", tag="yt")
                nc.sync.dma_start(yt, y_sb[qt], transpose=True)
                for n2 in range(D_MODEL // 512):
                    cp = nc.scalar.copy if (tail or n2 == 1) else nc.vector.tensor_copy
                    ops = ps_t.tile([128, 512], F32, name=f"op{qt}_{n2}", tag="tr")
                    nc.tensor.matmul(
                        ops,
                        lhsT=yt,
                        rhs=wo_sb[:, n2 * 512:(n2 + 1) * 512],
                        start=True,
                        stop=True,
                    )
                    ob = ob_pool.tile([128, 512], BF16, name=f"ob{qt}_{n2}", tag="ob")
                    cp(ob, ops)
                    nc.sync.dma_start(
                        out[qt * 128:(qt + 1) * 128, n2 * 512:(n2 + 1) * 512], ob
                    )

            # --- preamble ---
            if T > 1024:
                emit_xt_dma(2)
            warm = ps_t.tile([128, 512], F32, name="warmps", tag="tr")
            for i in range(34):
                nc.tensor.matmul(warm[:, 0:128], lhsT=trimask, rhs=trimask,
                                 start=True, stop=True)
            for g in range(2):
                for part in range(2):
                    emit_qkv_part(g, 0, part)
            emit_v_tile(0)

            # --- attention: k-tile PAIRS, heads interleaved ---
            bg = deque()
            op_pending = deque()

            for Q in range(QC):
                nkt = KPC * Q + KPC
                npair = nkt // 2
                c0 = Q * QW
                if Q + 3 < QC:
                    bg.append(lambda t=Q + 3: emit_xt_dma(t))
                for kt in range(KPC * Q, KPC * Q + KPC):
                    if kt > 0:
                        queue_v(kt)
                if Q + 1 < QC:
                    for g in range(2):
                        for part in range(2):
                            bg.append(
                                lambda g=g, t=Q + 1, p=part: emit_qkv_part(g, t, p)
                            )
                if Q > 0:
                    # drain the previous chunk's output projections NOW --
                    # deferring them all to the tail serializes ~30us
                    for s in range(SPC):
                        bg.append(
                            lambda qt=SPC * (Q - 1) + s: emit_outproj(qt)
                        )
                iters = npair
                if Q == QC - 1:
                    iters = max(1, (iters * 3) // 5)
                bg_total = len(bg)
                bg_emitted = 0

                av = [
                    ps_av.tile([128, 65 * SPC], F32, name=f"av{h}_{Q}",
                               tag=f"av{h}")
                    for h in range(HEADS_PER_CORE)
                ]
                # per-head PV output layout: h0 = [pv(64) | den], h1 = [den | pv(64)]
                # (h1's rhs slice starts at the shared ones column)

                def emit_epilogue(Q=Q, av=av):
                    for h in range(HEADS_PER_CORE):
                        den_off = 64 if h == 0 else 0
                        dat_off = 0 if h == 0 else 1
                        rec = rec_pool.tile([128, SPC], F32,
                                            name=f"rec{h}_{Q}", tag="rec")
                        nc.vector.reciprocal(
                            rec,
                            av[h][:, den_off:den_off + 65 * (SPC - 1) + 1:65],
                        )
                        for s in range(SPC):
                            qt = SPC * Q + s
                            nc.vector.tensor_scalar_mul(
                                y_sb[qt][:, h * 64:h * 64 + 64],
                                av[h][:, s * 65 + dat_off:s * 65 + dat_off + 64],
                                rec[:, s:s + 1],
                            )

                pend_h = [[], []]   # PV matmuls per head, 1-pair delay
                for p in range(npair):
                    kts = (2 * p, 2 * p + 1)
                    sps = [
                        ps_s.tile([128, 1024], F32, name=f"s{h}_{Q}_{p}",
                                  tag=f"s{h}")
                        for h in range(HEADS_PER_CORE)
                    ]
                    pts = []
                    # all scores matmuls adjacent, alternating heads: the
                    # K=64 matmuls land in different PE row groups so one
                    # head's LDWEIGHTS/drain hides under the other's matmul
                    for j, kt in enumerate(kts):
                        i = kt - KPC * Q
                        off = 128 * i if i > 0 else 0
                        for h in range(HEADS_PER_CORE):
                            hp = h * 64
                            nc.tensor.matmul(
                                sps[h][:, j * 512 + off:j * 512 + 512],
                                lhsT=k_sb[hp:hp + 64, kt * 128:(kt + 1) * 128],
                                rhs=q_sb[hp:hp + 64, c0 + off:c0 + 512],
                                start=True,
                                stop=True,
                            )
                    for h in range(HEADS_PER_CORE):
                        sp = sps[h]
                        pt = pt_pool.tile([128, 1024], BF16,
                                          name=f"pt{h}_{Q}_{p}", tag=f"pt{h}")
                        i0 = kts[0] - KPC * Q
                        if i0 >= 0:
                            # diagonal pair: separate exps per k-tile half
                            for j, kt in enumerate(kts):
                                off = 128 * (kt - KPC * Q)
                                nc.scalar.activation(
                                    pt[:, j * 512 + off:j * 512 + 512],
                                    sp[:, j * 512 + off:j * 512 + 512], EXP
                                )
                        else:
                            nc.scalar.activation(pt, sp, EXP)
                        pts.append(pt)
                        # PV(p-1) for this head runs on PE under this exp
                        for fn in pend_h[h]:
                            fn()
                        pend_h[h] = []
                    for h in range(HEADS_PER_CORE):
                        i0 = kts[0] - KPC * Q
                        if i0 >= 0:
                            for j, kt in enumerate(kts):
                                off = 128 * (kt - KPC * Q)
                                nc.gpsimd.tensor_mul(
                                    pts[h][:, j * 512 + off:j * 512 + off + 128],
                                    pts[h][:, j * 512 + off:j * 512 + off + 128],
                                    trimask,
                                )
                    for kt in kts:
                        ensure_v(kt)
                    for h in range(HEADS_PER_CORE):
                        rhs_off = 0 if h == 0 else 64
                        for j, kt in enumerate(kts):
                            i = kt - KPC * Q
                            for s in range(max(i, 0), SPC):
                                st = kt == 0 and s == max(i, 0)
                                sp_ = kt == nkt - 1 and s == SPC - 1
                                pend_h[h].append(
                                    lambda pt=pts[h], s=s, kt=kt, j=j, h=h,
                                    avt=av[h], ro=rhs_off, st=st, sp_=sp_:
                                    nc.tensor.matmul(
                                        avt[:, s * 65:s * 65 + 65],
                                        lhsT=pt[:, j * 512 + s * 128:
                                                j * 512 + s * 128 + 128],
                                        rhs=v_sb[kt][:, ro:ro + 65],
                                        start=st,
                                        stop=sp_,
                                    )
                                )
                    while bg and bg_emitted < (p + 1) * bg_total // iters:
                        bg.popleft()()
                        bg_emitted += 1
                for h in range(HEADS_PER_CORE):
                    for fn in pend_h[h]:
                        fn()
                    pend_h[h] = []
                emit_epilogue()
                if Q == QC - 1:
                    for s in range(SPC):
                        emit_outproj(SPC * Q + s, tail=True)
                while bg:
                    bg.popleft()()

    nc.compile()
    return nc


def make_in_maps(x, w_qkv, w_out, T: int = T_FULL):
    x = np.asarray(x, dtype=np.float32)
    w_qkv = np.asarray(w_qkv, dtype=np.float32)
    w_out = np.asarray(w_out, dtype=np.float32)
    xm = x.reshape(-1, D_MODEL)[:T]
    xt = np.ascontiguousarray(xm.T).astype(NPBF16)

    CH = D_MODEL // 128
    Wq = w_qkv[0:D_MODEL] * np.float32(1.0 / np.sqrt(HEAD_DIM))
    Wk = w_qkv[D_MODEL:2 * D_MODEL]
    Wv = w_qkv[2 * D_MODEL:3 * D_MODEL]

    in_maps = []
    for c in range(N_CORES):
        r0 = c * 128
        qk_rows = np.concatenate([Wq[r0:r0 + 128], Wk[r0:r0 + 128]], axis=0)
        qk_t = qk_rows.T.reshape(CH, 128, 256).transpose(1, 0, 2).reshape(128, CH * 256)
        v_rows = Wv[r0:r0 + 128]
        v_t = v_rows.T.reshape(CH, 128, 128).transpose(1, 0, 2).reshape(128, CH * 128)
        wo_t = np.ascontiguousarray(w_out[:, r0:r0 + 128].T)
        in_maps.append(
            {
                "xt": xt,
                "wqk": np.ascontiguousarray(qk_t).astype(NPBF16),
                "wv": np.ascontiguousarray(v_t).astype(NPBF16),
                "wo": wo_t.astype(NPBF16),
            }
        )
    return in_maps


_program_cache = {}


def get_program(T: int = T_FULL, loop_reps: int | None = None) -> bass.Bass:
    key = (T, loop_reps)
    if key not in _program_cache:
        _program_cache[key] = build_program(T, loop_reps)
    return _program_cache[key]


def run_on_hw(x, w_qkv, w_out, trace: bool = False, T: int = T_FULL):
    from concourse.bass_utils import run_bass_kernel_spmd

    nc = get_program(T)
    in_maps = make_in_maps(x, w_qkv, w_out, T)
    res = run_bass_kernel_spmd(nc, in_maps, core_ids=list(range(N_CORES)), trace=trace)
    acc = np.zeros((T, D_MODEL), np.float32)
    for c in range(N_CORES):
        acc += np.asarray(res.results[c]["out_partial"], dtype=np.float32)
    return acc.reshape(1, T, D_MODEL), res


def kernel(x, w_qkv, w_out):
    out, _ = run_on_hw(x, w_qkv, w_out)
    return out.astype(np.float32)
